# revision 3
# baseline (speedup 1.0000x reference)
"""CatAttention kernel for Trainium2, 8-core data-parallel over batch.

Computes (per the reference):
    pre = tanh(targ @ Wt^T + src @ Ws^T)        # [b, s, h]
    e   = pre @ V                               # [b, s]
    att = softmax(e, axis=s) * mask             # [b, s]
    ctx = einsum('bs,bsd->bd', att, src)
    out = ctx + targ
Returns (att, out).

Sharding: batch 32 -> 8 cores x 4 batches. W/V replicated.

Per-core layouts (host-prepared):
  srcT  [4, 512, 4096] f32   src transposed: d on partitions for the score matmul
  srcN  [4, 4096, 512] bf16  src natural: s on partitions for the ctx matmul
  wsT/wtT [512, 512] f32     W halves transposed ([d, h])
  targT [128, 16] f32        targ^T tiled: [p, 4*d_t + b]
  vcol  [128, 128] f32       V in cols {0,32,64,96}, zeros elsewhere
"""
import time
import numpy as np
import ml_dtypes

import concourse.bacc as bacc
import concourse.tile as tile
from concourse import mybir, masks
from concourse.bass_utils import run_bass_kernel_spmd

B, S, D = 32, 4096, 512
NCORES = 8
BPC = B // NCORES          # 4 batches per core
NT = D // 128              # 4 partition tiles of the hidden dims
SC = S // 512              # 8 score chunks (512 tokens each)
SN = S // 128              # 32 ctx tiles (128 tokens each)
F32 = mybir.dt.float32
BF16 = mybir.dt.bfloat16
AFT = mybir.ActivationFunctionType
AX = mybir.AxisListType

SRCT_BUFS = 24   # [128,512] f32 tiles (2KB/partition each)
SRCN_BUFS = 56   # [128,512] bf16 tiles (1KB/partition each)


def build(reps=1):
    nc = bacc.Bacc("TRN2", target_bir_lowering=False, debug=False)
    srcT = nc.dram_tensor("srcT", [BPC, D, S], F32, kind="ExternalInput").ap()
    srcN = nc.dram_tensor("srcN", [BPC, S, D], BF16, kind="ExternalInput").ap()
    wsT_d = nc.dram_tensor("wsT", [D, D], F32, kind="ExternalInput").ap()
    wtT_d = nc.dram_tensor("wtT", [D, D], F32, kind="ExternalInput").ap()
    targT_d = nc.dram_tensor("targT", [128, NT * BPC], F32, kind="ExternalInput").ap()
    targ4_d = nc.dram_tensor("targ4", [BPC, D], F32, kind="ExternalInput").ap()
    vcol_d = nc.dram_tensor("vcol", [128, 128], F32, kind="ExternalInput").ap()
    mask_d = nc.dram_tensor("maskd", [BPC, S], F32, kind="ExternalInput").ap()
    attw = nc.dram_tensor("attw", [BPC, S], F32, kind="ExternalOutput").ap()
    hid = nc.dram_tensor("hid", [BPC, D], F32, kind="ExternalOutput").ap()

    with tile.TileContext(nc) as tc:
        with tc.tile_pool(name="const", bufs=1) as const, \
             tc.tile_pool(name="work", bufs=1) as work:
            # ---- constants ----
            wsT_sb = const.tile([128, NT * D], F32, tag="wsT")
            wtT_sb = const.tile([128, NT * D], F32, tag="wtT")
            for dt_ in range(NT):
                nc.sync.dma_start(out=wsT_sb[:, D * dt_:D * (dt_ + 1)],
                                  in_=wsT_d[128 * dt_:128 * (dt_ + 1), :])
                nc.sync.dma_start(out=wtT_sb[:, D * dt_:D * (dt_ + 1)],
                                  in_=wtT_d[128 * dt_:128 * (dt_ + 1), :])
            targT_sb = const.tile([128, NT * BPC], F32, tag="targT")
            nc.sync.dma_start(out=targT_sb[:], in_=targT_d[:])
            vcol_sb = const.tile([128, 128], F32, tag="vcol")
            nc.sync.dma_start(out=vcol_sb[:], in_=vcol_d[:])
            targ_sb = const.tile([128, D], F32, tag="targ")
            for b in range(BPC):
                nc.sync.dma_start(out=targ_sb[32 * b:32 * b + 1, :],
                                  in_=targ4_d[b:b + 1, :])
            mask_sb = const.tile([128, S], F32, tag="mask")
            nc.gpsimd.memset(mask_sb[:], 1.0)
            for b in range(BPC):
                nc.sync.dma_start(out=mask_sb[32 * b:32 * b + 1, :],
                                  in_=mask_d[b:b + 1, :])
            ident = const.tile([128, 128], F32, tag="ident")
            masks.make_identity(nc, ident[:])

            for rep in range(reps):
                # ---- per-rep working buffers ----
                e_sb = work.tile([128, S], F32, tag="e")
                attT_sb = work.tile([128, S], BF16, tag="attT")
                tp_sb = work.tile([128, NT * BPC], F32, tag="tp")
                negm = work.tile([128, 1], F32, tag="negm")
                ssum = work.tile([128, 1], F32, tag="ssum")
                rins = work.tile([128, 1], F32, tag="rins")
                out_sb = work.tile([128, D], F32, tag="out")

                # ================= phase A: scores =================
                with tc.tile_pool(name="psA", bufs=1, space="PSUM") as psA, \
                     tc.tile_pool(name="stp", bufs=SRCT_BUFS) as stp, \
                     tc.tile_pool(name="prep", bufs=16) as prep:
                    # targ projection tp[h, 4*h_t + b]
                    tp_ps = psA.tile([128, NT * BPC], F32, tag="tp")
                    for ht in range(NT):
                        for dt_ in range(NT):
                            nc.tensor.matmul(
                                out=tp_ps[:, 4 * ht:4 * ht + 4],
                                lhsT=wtT_sb[:, D * dt_ + 128 * ht:D * dt_ + 128 * (ht + 1)],
                                rhs=targT_sb[:, 4 * dt_:4 * dt_ + 4],
                                start=(dt_ == 0), stop=(dt_ == NT - 1))
                    nc.scalar.copy(out=tp_sb[:], in_=tp_ps[:])

                    for c in range(SC):
                        st = {}
                        for b in range(BPC):
                            for dt_ in range(NT):
                                t = stp.tile([128, 512], F32, tag="st")
                                nc.sync.dma_start(
                                    out=t[:],
                                    in_=srcT[b, 128 * dt_:128 * (dt_ + 1),
                                             512 * c:512 * (c + 1)])
                                st[(b, dt_)] = t
                        e4 = psA.tile([128, 512], F32, tag="e4", bufs=2)
                        for b in range(BPC):
                            pre_t = {}
                            for ht in range(NT):
                                spp = psA.tile([128, 512], F32, tag="sp", bufs=3)
                                for dt_ in range(NT):
                                    nc.tensor.matmul(
                                        out=spp[:],
                                        lhsT=wsT_sb[:, D * dt_ + 128 * ht:D * dt_ + 128 * (ht + 1)],
                                        rhs=st[(b, dt_)][:],
                                        start=(dt_ == 0), stop=(dt_ == NT - 1))
                                pre = prep.tile([128, 512], F32, tag="pre")
                                nc.scalar.activation(
                                    out=pre[:], in_=spp[:], func=AFT.Tanh,
                                    bias=tp_sb[:, 4 * ht + b:4 * ht + b + 1])
                                pre_t[ht] = pre
                            for ht in range(NT):
                                nc.tensor.matmul(
                                    out=e4[32 * b:32 * (b + 1), :],
                                    lhsT=vcol_sb[:, 32 * ht:32 * (ht + 1)],
                                    rhs=pre_t[ht][:],
                                    start=(ht == 0), stop=(ht == NT - 1),
                                    tile_position=(0, 32 * b))
                        nc.scalar.copy(out=e_sb[:, 512 * c:512 * (c + 1)], in_=e4[:])

                # ================= phase B: softmax + transpose =================
                nc.vector.reduce_max(out=negm[:], in_=e_sb[:], axis=AX.X, negate=True)
                nc.scalar.activation(out=e_sb[:], in_=e_sb[:], func=AFT.Exp,
                                     bias=negm[:, 0:1], accum_out=ssum[:])
                nc.vector.reciprocal(out=rins[:], in_=ssum[:])
                nc.vector.tensor_scalar_mul(e_sb[:], e_sb[:], rins[:, 0:1])
                nc.vector.tensor_mul(e_sb[:], e_sb[:], mask_sb[:])
                for b in range(BPC):
                    nc.sync.dma_start(out=attw[b:b + 1, :],
                                      in_=e_sb[32 * b:32 * b + 1, :])

                with tc.tile_pool(name="psB", bufs=1, space="PSUM") as psB, \
                     tc.tile_pool(name="snp", bufs=SRCN_BUFS) as snp:
                    for k in range(SC):
                        tpp = psB.tile([128, 512], F32, tag="tpp", bufs=2)
                        for q in range(4):
                            nc.tensor.transpose(
                                tpp[:, 128 * q:128 * (q + 1)],
                                e_sb[:, 128 * (4 * k + q):128 * (4 * k + q + 1)],
                                ident[:])
                        nc.scalar.copy(out=attT_sb[:, 512 * k:512 * (k + 1)],
                                       in_=tpp[:])

                    # ================= phase C: context =================
                    ctx = psB.tile([128, 512], F32, tag="ctx")
                    for b in range(BPC):
                        for cn in range(SN):
                            sn = snp.tile([128, 512], BF16, tag="sn")
                            nc.sync.dma_start(
                                out=sn[:],
                                in_=srcN[b, 128 * cn:128 * (cn + 1), :])
                            nc.tensor.matmul(
                                out=ctx[32 * b:32 * (b + 1), :],
                                lhsT=attT_sb[:, 128 * cn + 32 * b:128 * cn + 32 * (b + 1)],
                                rhs=sn[:],
                                start=(cn == 0), stop=(cn == SN - 1),
                                tile_position=(0, 32 * b))
                        nc.vector.tensor_add(out_sb[32 * b:32 * b + 1, :],
                                             ctx[32 * b:32 * b + 1, :],
                                             targ_sb[32 * b:32 * b + 1, :])
                        nc.sync.dma_start(out=hid[b:b + 1, :],
                                          in_=out_sb[32 * b:32 * b + 1, :])
    nc.compile()
    return nc


def shard_inputs(hidden_targ, hidden_src, mask, W, V):
    """Build the 8 per-core input maps."""
    hidden_targ = np.asarray(hidden_targ, dtype=np.float32)
    hidden_src = np.asarray(hidden_src, dtype=np.float32)
    mask = np.asarray(mask, dtype=np.float32)
    W = np.asarray(W, dtype=np.float32)
    V = np.asarray(V, dtype=np.float32)

    wsT = np.ascontiguousarray(W[:, D:].T)
    wtT = np.ascontiguousarray(W[:, :D].T)
    vcol = np.zeros((128, 128), dtype=np.float32)
    vcol[:, 0:128:32] = V.reshape(NT, 128).T

    in_maps = []
    for c in range(NCORES):
        bsl = slice(BPC * c, BPC * (c + 1))
        src_c = hidden_src[bsl]
        targ_c = hidden_targ[bsl]
        in_maps.append({
            "srcT": np.ascontiguousarray(src_c.transpose(0, 2, 1)),
            "srcN": src_c.astype(ml_dtypes.bfloat16),
            "wsT": wsT,
            "wtT": wtT,
            "targT": np.ascontiguousarray(
                targ_c.T.reshape(NT, 128, BPC).transpose(1, 0, 2).reshape(128, NT * BPC)),
            "targ4": np.ascontiguousarray(targ_c),
            "vcol": vcol,
            "maskd": np.ascontiguousarray(mask[bsl]),
        })
    return in_maps


_CACHE = {}


def get_built(reps=1):
    if reps not in _CACHE:
        _CACHE[reps] = build(reps)
    return _CACHE[reps]


def run(in_maps, reps=1):
    nc = get_built(reps)
    return run_bass_kernel_spmd(nc, in_maps, list(range(NCORES)))


def kernel(hidden_targ, hidden_src, mask, W, V):
    in_maps = shard_inputs(hidden_targ, hidden_src, mask, W, V)
    res = run(in_maps, reps=1)
    att = np.concatenate([res.results[c]["attw"] for c in range(NCORES)], axis=0)
    out = np.concatenate([res.results[c]["hid"] for c in range(NCORES)], axis=0)
    return att, out


# revision 27
# speedup vs baseline: 386.6906x; 386.6906x over previous
"""CatAttention kernel for Trainium2, 8-core data-parallel over batch.

Computes (per the reference):
    pre = tanh(targ @ Wt^T + src @ Ws^T)        # [b, s, h]
    e   = pre @ V                               # [b, s]
    att = softmax(e, axis=s) * mask             # [b, s]
    ctx = einsum('bs,bsd->bd', att, src)
    out = ctx + targ
Returns (att, out).

Sharding: batch 32 -> 8 cores x 4 batches. W/V replicated.

Per-core layouts (host-prepared):
  srcT  [4, 512, 4096] f32   src transposed: d on partitions for the score matmul
  srcN  [4, 4096, 512] bf16  src natural: s on partitions for the ctx matmul
  wsT/wtT [512, 512] f32     W halves transposed ([d, h])
  targT [128, 16] f32        targ^T tiled: [p, 4*d_t + b]
  vcol  [128, 128] f32       V in cols {0,32,64,96}, zeros elsewhere
"""
import numpy as np
import ml_dtypes

import concourse.bacc as bacc
import concourse.tile as tile
from concourse import mybir, masks
from concourse.bass_utils import run_bass_kernel_spmd

B, S, D = 32, 4096, 512
NCORES = 8
BPC = B // NCORES          # 4 batches per core
NT = D // 128              # 4 partition tiles of the hidden dims
SC = S // 512              # 8 score chunks (512 tokens each)
SN = S // 128              # 32 ctx tiles (128 tokens each)
F32 = mybir.dt.float32
F32R = mybir.dt.float32r   # relaxed-precision fp32: 1 cycle/row on PE (vs 4 for fp32)
BF16 = mybir.dt.bfloat16
AFT = mybir.ActivationFunctionType
AX = mybir.AxisListType


def _r(ap):
    return ap.bitcast(F32R)

SRCT_BUFS = 12   # [128, 1024] f32 tiles (4KB/partition each)
SRCN_BUFS = 12   # [128, 2048] bf16 tiles (4KB/partition each)


def _body(nc, tc, io, pools):
    """One full forward pass over this core's 4 batches."""
    srcT, srcN, attw, hid = io["srcT"], io["srcN"], io["attw"], io["hid"]
    const_t, work, ps, stp, prep, snp = pools
    (wsT_sb, wtT_half, targT_sb, vcol_sb, targ_sb, mask_sb, ident) = const_t

    e_sb = work.tile([128, S], F32, tag="e")
    attT_sb = work.tile([128, S], BF16, tag="attT")
    tp_sb = work.tile([128, NT * BPC], F32, tag="tp")
    negm = work.tile([128, 1], F32, tag="negm")
    mx8 = work.tile([128, SC], F32, tag="mx8")
    ssum = work.tile([128, 1], F32, tag="ssum")
    rins = work.tile([128, 1], F32, tag="rins")
    out_sb = work.tile([128, D], F32, tag="out")

    # ---- targ projection tp[h, 4*h_t + b] ----
    tp_ps = ps.tile([128, NT * BPC], F32, tag="ctx", bufs=1)
    for ht in range(NT):
        for dt_ in range(NT):
            nc.tensor.matmul(
                out=tp_ps[:, 4 * ht:4 * ht + 4],
                lhsT=wtT_half[dt_ // 2][:, D * (dt_ % 2) + 128 * ht:D * (dt_ % 2) + 128 * (ht + 1)],
                rhs=targT_sb[:, 4 * dt_:4 * dt_ + 4],
                start=(dt_ == 0), stop=(dt_ == NT - 1))
    nc.scalar.copy(out=tp_sb[:], in_=tp_ps[:])

    # ================= phase A: scores =================
    for c in range(SC):
        st = {}
        for half in range(2):
            for dt_ in range(NT):
                t = stp.tile([128, 2 * 512], F32R, tag="st")
                nc.sync.dma_start(
                    out=t[:],
                    in_=srcT[2 * half:2 * half + 2, 128 * dt_:128 * (dt_ + 1),
                             512 * c:512 * (c + 1)].rearrange("b p s -> p b s"))
                st[(dt_, half)] = t
        e4 = ps.tile([128, 512], F32, tag="e4", bufs=2)
        pre_t = {}
        for b in range(BPC):
            for ht in range(NT):
                spp = ps.tile([128, 512], F32, tag="sp", bufs=4)
                for dt_ in range(NT):
                    nc.tensor.matmul(
                        out=spp[:],
                        lhsT=wsT_sb[:, D * dt_ + 128 * ht:D * dt_ + 128 * (ht + 1)],
                        rhs=st[(dt_, b // 2)][:, 512 * (b % 2):512 * (b % 2 + 1)],
                        start=(dt_ == 0), stop=(dt_ == NT - 1))
                pre = prep.tile([128, 512], F32R, tag="pre")
                nc.scalar.activation(
                    out=pre[:], in_=spp[:], func=AFT.Tanh,
                    bias=tp_sb[:, 4 * ht + b:4 * ht + b + 1])
                pre_t[(b, ht)] = pre
        for b in range(BPC):
            for ht in range(NT):
                nc.tensor.matmul(
                    out=e4[:],
                    lhsT=vcol_sb[:, 128 * (4 * ht + b):128 * (4 * ht + b + 1)],
                    rhs=pre_t[(b, ht)][:],
                    start=(b == 0 and ht == 0), stop=(b == BPC - 1 and ht == NT - 1),
                    skip_group_check=True)
        nc.vector.tensor_copy(e_sb[:, 512 * c:512 * (c + 1)], e4[:])
        nc.vector.reduce_max(out=mx8[:, c:c + 1], in_=e_sb[:, 512 * c:512 * (c + 1)],
                             axis=AX.X)

    # ================= phase B: softmax + transpose =================
    nc.vector.reduce_max(out=negm[:], in_=mx8[:], axis=AX.X, negate=True)
    nc.scalar.activation(out=e_sb[:], in_=e_sb[:], func=AFT.Exp,
                         bias=negm[:, 0:1], accum_out=ssum[:])
    nc.vector.reciprocal(out=rins[:], in_=ssum[:])
    for k in range(SC):
        sl = slice(512 * k, 512 * (k + 1))
        nc.vector.scalar_tensor_tensor(
            out=e_sb[:, sl], in0=e_sb[:, sl], scalar=rins[:, 0:1],
            in1=mask_sb[:, sl], op0=mybir.AluOpType.mult, op1=mybir.AluOpType.mult)
        tpp = ps.tile([128, 512], F32, tag="tpp", bufs=1)
        for q in range(4):
            nc.tensor.transpose(
                tpp[:, 128 * q:128 * (q + 1)],
                e_sb[:, 128 * (4 * k + q):128 * (4 * k + q + 1)],
                ident[:])
        nc.scalar.copy(out=attT_sb[:, 512 * k:512 * (k + 1)], in_=tpp[:])
    for b in range(BPC):
        nc.sync.dma_start(out=attw[b:b + 1, :], in_=e_sb[32 * b:32 * b + 1, :])

    # ================= phase C: context =================
    for b in range(BPC):
        ctx = ps.tile([128, 512], F32, tag="ctx", bufs=1)
        for g in range(SN // 4):
            sn = snp.tile([128, 4 * 512], BF16, tag="sn")
            eng = nc.gpsimd if g % 2 == 0 else nc.sync
            eng.dma_start(
                out=sn[:],
                in_=srcN[b, 512 * g:512 * (g + 1), :].rearrange(
                    "(c p) d -> p c d", p=128))
            for q in range(4):
                cn = 4 * g + q
                # Full [128,128] attT block as stationary: row 32b of the
                # output gets batch b's context; other rows get junk that is
                # never read.
                nc.tensor.matmul(
                    out=ctx[:],
                    lhsT=attT_sb[:, 128 * cn:128 * (cn + 1)],
                    rhs=sn[:, 512 * q:512 * (q + 1)],
                    start=(cn == 0), stop=(cn == SN - 1))
        nc.vector.tensor_add(out_sb[32 * b:32 * b + 1, :],
                             ctx[32 * b:32 * b + 1, :],
                             targ_sb[32 * b:32 * b + 1, :])
        nc.sync.dma_start(out=hid[b:b + 1, :], in_=out_sb[32 * b:32 * b + 1, :])


def build(reps=1):
    nc = bacc.Bacc("TRN2", target_bir_lowering=False, debug=False)
    io = {
        "srcT": nc.dram_tensor("srcT", [BPC, D, S], F32R, kind="ExternalInput").ap(),
        "srcN": nc.dram_tensor("srcN", [BPC, S, D], BF16, kind="ExternalInput").ap(),
        "attw": nc.dram_tensor("attw", [BPC, S], F32, kind="ExternalOutput").ap(),
        "hid": nc.dram_tensor("hid", [BPC, D], F32, kind="ExternalOutput").ap(),
    }
    wsT_d = nc.dram_tensor("wsT", [D, D], F32R, kind="ExternalInput").ap()
    wtT_d = nc.dram_tensor("wtT", [D, D], F32R, kind="ExternalInput").ap()
    targT_d = nc.dram_tensor("targT", [128, NT * BPC], F32R, kind="ExternalInput").ap()
    targ4_d = nc.dram_tensor("targ4", [BPC, D], F32, kind="ExternalInput").ap()
    vcol_d = nc.dram_tensor("vcol", [128, NT * BPC * 128], F32R, kind="ExternalInput").ap()
    mask_d = nc.dram_tensor("maskd", [BPC, S], F32, kind="ExternalInput").ap()

    with tile.TileContext(nc) as tc:
        with tc.tile_pool(name="const", bufs=1) as const, \
             tc.tile_pool(name="work", bufs=1) as work, \
             tc.tile_pool(name="ps", bufs=1, space="PSUM") as ps, \
             tc.tile_pool(name="stp", bufs=SRCT_BUFS) as stp, \
             tc.tile_pool(name="prep", bufs=20) as prep, \
             tc.tile_pool(name="snp", bufs=SRCN_BUFS) as snp:
            # ---- constants ----
            wsT_sb = const.tile([128, NT * D], F32R, tag="wsT")
            wtT_sb = const.tile([128, NT * D], F32R, tag="wtT")
            wtT_half = [wtT_sb[:, :NT * D // 2], wtT_sb[:, NT * D // 2:]]
            targT_sb = const.tile([128, NT * BPC], F32R, tag="targT")
            nc.scalar.dma_start(out=targT_sb[:], in_=targT_d[:])
            for dt_ in range(NT):
                nc.scalar.dma_start(
                    out=wtT_half[dt_ // 2][:, D * (dt_ % 2):D * (dt_ % 2 + 1)],
                    in_=wtT_d[128 * dt_:128 * (dt_ + 1), :])
            for dt_ in range(NT):
                nc.scalar.dma_start(out=wsT_sb[:, D * dt_:D * (dt_ + 1)],
                                    in_=wsT_d[128 * dt_:128 * (dt_ + 1), :])
            vcol_sb = const.tile([128, NT * BPC * 128], F32R, tag="vcol")
            nc.scalar.dma_start(out=vcol_sb[:], in_=vcol_d[:])
            targ_sb = const.tile([128, D], F32, tag="targ")
            for b in range(BPC):
                nc.scalar.dma_start(out=targ_sb[32 * b:32 * b + 1, :],
                                    in_=targ4_d[b:b + 1, :])
            mask_sb = const.tile([128, S], F32, tag="mask")
            nc.gpsimd.memset(mask_sb[:], 1.0)
            for b in range(BPC):
                nc.scalar.dma_start(out=mask_sb[32 * b:32 * b + 1, :],
                                    in_=mask_d[b:b + 1, :])
            ident = const.tile([128, 128], F32, tag="ident")
            masks.make_identity(nc, ident[:])
            const_t = (wsT_sb, wtT_half, targT_sb, vcol_sb, targ_sb, mask_sb, ident)
            pools = (const_t, work, ps, stp, prep, snp)

            if reps == 1:
                _body(nc, tc, io, pools)
            else:
                with tc.For_i(0, reps, 1):
                    _body(nc, tc, io, pools)
    nc.compile()
    return nc


def shard_inputs(hidden_targ, hidden_src, mask, W, V):
    """Build the 8 per-core input maps."""
    hidden_targ = np.asarray(hidden_targ, dtype=np.float32)
    hidden_src = np.asarray(hidden_src, dtype=np.float32)
    mask = np.asarray(mask, dtype=np.float32)
    W = np.asarray(W, dtype=np.float32)
    V = np.asarray(V, dtype=np.float32)

    wsT = np.ascontiguousarray(W[:, D:].T)
    wtT = np.ascontiguousarray(W[:, :D].T)
    vcol = np.zeros((128, NT * BPC * 128), dtype=np.float32)
    for ht in range(NT):
        for b in range(BPC):
            vcol[:, 128 * (4 * ht + b) + 32 * b] = V[128 * ht:128 * (ht + 1)]

    in_maps = []
    for c in range(NCORES):
        bsl = slice(BPC * c, BPC * (c + 1))
        src_c = hidden_src[bsl]
        targ_c = hidden_targ[bsl]
        in_maps.append({
            "srcT": np.ascontiguousarray(src_c.transpose(0, 2, 1)),
            "srcN": src_c.astype(ml_dtypes.bfloat16),
            "wsT": wsT,
            "wtT": wtT,
            "targT": np.ascontiguousarray(
                targ_c.T.reshape(NT, 128, BPC).transpose(1, 0, 2).reshape(128, NT * BPC)),
            "targ4": np.ascontiguousarray(targ_c),
            "vcol": vcol,
            "maskd": np.ascontiguousarray(mask[bsl]),
        })
    return in_maps


_CACHE = {}


def get_built(reps=1):
    if reps not in _CACHE:
        _CACHE[reps] = build(reps)
    return _CACHE[reps]


def run(in_maps, reps=1):
    nc = get_built(reps)
    return run_bass_kernel_spmd(nc, in_maps, list(range(NCORES)))


def kernel(hidden_targ, hidden_src, mask, W, V):
    in_maps = shard_inputs(hidden_targ, hidden_src, mask, W, V)
    res = run(in_maps, reps=1)
    att = np.concatenate([res.results[c]["attw"] for c in range(NCORES)], axis=0)
    out = np.concatenate([res.results[c]["hid"] for c in range(NCORES)], axis=0)
    return att, out
